# revision 6
# baseline (speedup 1.0000x reference)
"""Trainium2 Bass kernel for DecoderOnlyAspire segment-reduce problem.

Computes, for hidden [B=8, S=4096, D=1536] f32:
  - doc_reps  [B, D]    : last-token pooling (per reference semantics)
  - sent_reps [B, M, D] : per-sentence segment means (M = 24)

Strategy: data-parallel over batch across 8 NeuronCores (one example per
core; no cross-core communication).  On each core both outputs come from
PE matmuls: out[M+1, D] = W.T @ hidden_b where W [S, M+1] is the exact
{0,1} one-hot of sent_ids (col M = last-token indicator); the division
by segment count is folded into a per-partition scalar multiply on the
way out of PSUM.

To hit the memory roofline, the host losslessly recodes hidden into a
bf16 hi/lo pair (hi = bf16(h), lo = bf16(h - hi); |h - hi - lo| <=
~2^-18 |h|) laid out so every DMA reads large contiguous per-partition
chunks, and the PE runs bf16 matmuls (1 cycle/row instead of fp32's 4).
Token t lives at partition p = t // 32, slot q = t % 32; W rows are
permuted identically, so the matmul contraction stays consistent.
"""

import os

import numpy as np

B, S, D, M = 8, 4096, 1536, 24
P = 128              # SBUF partitions
Q = S // P           # 32 token slots per partition
MM = M + 1           # segment columns + doc (last-token) column
NBANK = 512          # fp32 elements per PSUM bank
NJ = D // NBANK      # 3 bank-column chunks
# Token-slot chunk sizes per DMA: big in the middle for DMA efficiency,
# small at the end so the PE trail after the last DMA is short.
CHUNKS = [8, 8, 8, 4, 2, 1, 1]
assert sum(CHUNKS) == Q

_PROGRAM = None
_LAST_RESULTS = None  # BassKernelResults of the most recent run (for test harness)


def _build_program():
    import concourse.bacc as bacc
    import concourse.tile as tile
    from concourse import mybir

    nc = bacc.Bacc("TRN2", target_bir_lowering=False, debug=False)

    # [p, q, a, d] bf16: a=0 hi, a=1 lo; token t = p*Q + q
    h_in = nc.declare_dram_parameter(
        "hidden_b", [P, Q * 2 * D], mybir.dt.bfloat16, isOutput=False
    )
    w_in = nc.declare_dram_parameter("w_b", [P, Q * MM], mybir.dt.bfloat16, isOutput=False)
    inv_in = nc.declare_dram_parameter("inv_b", [MM, 1], mybir.dt.float32, isOutput=False)
    out_ext = nc.declare_dram_parameter("out_b", [MM, D], mybir.dt.float32, isOutput=True)

    h_view = h_in[:].rearrange("p (q a d) -> p q a d", q=Q, a=2)

    with tile.TileContext(nc) as tc:
        with (
            tc.tile_pool(name="w", bufs=1) as wpool,
            tc.tile_pool(name="h", bufs=3) as hpool,
            tc.tile_pool(name="ps", bufs=1, space="PSUM") as pspool,
            tc.tile_pool(name="o", bufs=1) as opool,
        ):
            # W + inv ride the ACT HWDGE ring so they don't delay the first
            # hidden chunk on the SP ring.
            w_tile = wpool.tile([P, Q, MM], mybir.dt.bfloat16)
            nc.scalar.dma_start(w_tile[:], w_in[:].rearrange("p (q m) -> p q m", q=Q))
            inv_tile = wpool.tile([MM, 1], mybir.dt.float32, tag="inv")
            nc.scalar.dma_start(inv_tile[:], inv_in[:])

            psum_t = pspool.tile([MM, D], mybir.dt.float32)
            tg0 = 0
            for qc in CHUNKS:
                h_t = hpool.tile([P, qc, 2, D], mybir.dt.bfloat16, tag="h")
                nc.sync.dma_start(h_t[:], h_view[:, tg0 : tg0 + qc, :, :])
                for q in range(qc):
                    tg = tg0 + q
                    for a in range(2):
                        for j in range(NJ):
                            nc.tensor.matmul(
                                psum_t[:, j * NBANK : (j + 1) * NBANK],
                                w_tile[:, tg, :],
                                h_t[:, q, a, j * NBANK : (j + 1) * NBANK],
                                start=(tg == 0 and a == 0),
                                stop=(tg == Q - 1 and a == 1),
                            )
                tg0 += qc

            # Per-bank epilogue: each bank's scale+store chases its own
            # accumulation group instead of waiting for all three.
            out_t = opool.tile([MM, D], mybir.dt.float32)
            for j in range(NJ):
                sl = slice(j * NBANK, (j + 1) * NBANK)
                nc.vector.tensor_scalar_mul(out_t[:, sl], psum_t[:, sl], inv_tile[:, 0:1])
                nc.sync.dma_start(out_ext[:, sl], out_t[:, sl])

    nc.compile()
    return nc


def _get_program():
    global _PROGRAM
    if _PROGRAM is None:
        _PROGRAM = _build_program()
    return _PROGRAM


def _prepare_inputs(hidden, attn_mask, sent_ids):
    """Host-side lossless recode: bf16 hi/lo split + layout permute + W."""
    import ml_dtypes

    bf16 = ml_dtypes.bfloat16

    # Last-token index per example (same semantics as the reference).
    left_padding = int(attn_mask[:, -1].sum()) == B
    seq_lengths = attn_mask.sum(axis=1) - 1  # [B]
    if left_padding:
        idx = np.full(B, S - 1, dtype=np.int64)
    else:
        idx = seq_lengths.astype(np.int64)

    hi = hidden.astype(bf16)
    lo = (hidden - hi.astype(np.float32)).astype(bf16)
    # [B, S, D] -> [B, P, Q, 2, D] with token t = p*Q + q
    pair = np.stack([hi, lo], axis=2)  # [B, S, 2, D]
    h_dev = np.ascontiguousarray(
        pair.reshape(B, P, Q, 2, D).reshape(B, P, Q * 2 * D)
    )

    w = np.zeros((B, S, MM), dtype=bf16)
    tok = np.arange(S)
    inv = np.zeros((B, MM, 1), dtype=np.float32)
    for b in range(B):
        w[b, tok, sent_ids[b]] = 1
        w[b, idx[b], M] = 1
        counts = np.bincount(sent_ids[b], minlength=M)
        inv[b, :M, 0] = (1.0 / np.maximum(counts, 1)).astype(np.float32)
        inv[b, M, 0] = 1.0
    w_dev = np.ascontiguousarray(w.reshape(B, P, Q * MM))
    return h_dev, w_dev, inv


def kernel(hidden, attn_mask, sent_ids, max_sents):
    global _LAST_RESULTS
    from concourse.bass_utils import run_bass_kernel_spmd

    hidden = np.ascontiguousarray(np.asarray(hidden, dtype=np.float32))
    attn_mask = np.asarray(attn_mask).astype(np.int32)
    sent_ids = np.asarray(sent_ids).astype(np.int32)
    m = int(max_sents)
    assert hidden.shape == (B, S, D) and m == M

    h_dev, w_dev, inv = _prepare_inputs(hidden, attn_mask, sent_ids)

    nc = _get_program()
    in_maps = [
        {"hidden_b": h_dev[b], "w_b": w_dev[b], "inv_b": inv[b]} for b in range(B)
    ]
    trace = bool(os.environ.get("KERNEL_TRACE"))
    kwargs = {}
    if trace:
        base = os.environ.get("KERNEL_TRACE_DIR")
        if base:
            import tempfile

            os.makedirs(base, exist_ok=True)
            kwargs["tmpdir"] = tempfile.mkdtemp(dir=base)
        if os.environ.get("KERNEL_TRACE_CORES"):
            kwargs["trace_cores"] = [
                int(c) for c in os.environ["KERNEL_TRACE_CORES"].split(",")
            ]
    res = run_bass_kernel_spmd(nc, in_maps, list(range(B)), trace=trace, **kwargs)
    _LAST_RESULTS = res
    out = np.stack([res.results[b]["out_b"] for b in range(B)])  # [B, MM, D]
    doc_reps = out[:, M, :].copy()
    sent_reps = out[:, :M, :].copy()
    return doc_reps, sent_reps


# revision 7
# speedup vs baseline: 1.1131x; 1.1131x over previous
"""Trainium2 Bass kernel for DecoderOnlyAspire segment-reduce problem.

Computes, for hidden [B=8, S=4096, D=1536] f32:
  - doc_reps  [B, D]    : last-token pooling (per reference semantics)
  - sent_reps [B, M, D] : per-sentence segment means (M = 24)

Strategy: data-parallel over batch across 8 NeuronCores (one example per
core; no cross-core communication).  On each core both outputs come from
PE matmuls: out[M+1, D] = W.T @ hidden_b where W [S, M+1] is the exact
{0,1} one-hot of sent_ids (col M = last-token indicator); the division
by segment count is folded into a per-partition scalar multiply on the
way out of PSUM.

Memory-roofline design: the host losslessly recodes hidden into a bf16
hi/lo pair (hi = bf16(h), lo = bf16(h - hi); |h - hi - lo| <= ~2^-18
|h|, same total bytes as fp32) packed so every DMA reads large
contiguous per-partition chunks at near line rate, and the PE runs bf16
matmuls (1 cycle/row instead of fp32's 4).  Token t lives at partition
p = t // 32, slot q = t % 32; W rows are permuted identically.  W is
inlined at the head of chunk 0 of the same stream so no small-packet
DMA ever stalls the pipeline; chunk sizes taper at the end so the PE
trail after the last DMA is short.
"""

import os

import numpy as np

B, S, D, M = 8, 4096, 1536, 24
P = 128              # SBUF partitions
Q = S // P           # 32 token slots per partition
MM = M + 1           # segment columns + doc (last-token) column
NBANK = 512          # fp32 elements per PSUM bank
NJ = D // NBANK      # 3 bank-column chunks
WCOLS = Q * MM       # 800 bf16 of inlined W per partition
TOK = 2 * D          # bf16 elements per token slot (hi+lo)

QC0 = 2                         # token slots sharing chunk 0 with W
CHUNKS = [8, 8, 8, 3, 2, 1]     # remaining token-slot chunks
assert QC0 + sum(CHUNKS) == Q

_PROGRAM = None
_LAST_RESULTS = None  # BassKernelResults of the most recent run (for test harness)


def _build_program():
    import concourse.bacc as bacc
    import concourse.tile as tile
    from concourse import mybir

    nc = bacc.Bacc("TRN2", target_bir_lowering=False, debug=False)

    # Per partition p: [ W_p (800 bf16) | token slots q=0..31, each hi|lo (2*1536) ]
    data_in = nc.declare_dram_parameter(
        "data_b", [P, WCOLS + Q * TOK], mybir.dt.bfloat16, isOutput=False
    )
    inv_in = nc.declare_dram_parameter("inv_b", [MM, 1], mybir.dt.float32, isOutput=False)
    out_ext = nc.declare_dram_parameter("out_b", [MM, D], mybir.dt.float32, isOutput=True)

    with tile.TileContext(nc) as tc:
        with (
            tc.tile_pool(name="w", bufs=1) as wpool,
            tc.tile_pool(name="h", bufs=2) as hpool,
            tc.tile_pool(name="hs", bufs=3) as hspool,
            tc.tile_pool(name="ps", bufs=1, space="PSUM") as pspool,
            tc.tile_pool(name="o", bufs=1) as opool,
        ):
            # inv rides the ACT HWDGE ring; it is only needed at the end.
            inv_tile = wpool.tile([MM, 1], mybir.dt.float32, tag="inv")
            nc.scalar.dma_start(inv_tile[:], inv_in[:])

            # Chunk 0: W + first QC0 token slots, resident for the whole kernel.
            c0 = wpool.tile([P, WCOLS + QC0 * TOK], mybir.dt.bfloat16, tag="c0")
            nc.sync.dma_start(c0[:], data_in[:, : WCOLS + QC0 * TOK])
            w_view = c0[:, :WCOLS].rearrange("p (q m) -> p q m", q=Q)
            h0_view = c0[:, WCOLS:].rearrange("p (q a d) -> p q a d", q=QC0, a=2)

            psum_t = pspool.tile([MM, D], mybir.dt.float32)

            def mms(tg, h_slice_of):
                for a in range(2):
                    for j in range(NJ):
                        nc.tensor.matmul(
                            psum_t[:, j * NBANK : (j + 1) * NBANK],
                            w_view[:, tg, :],
                            h_slice_of(a, j),
                            start=(tg == 0 and a == 0),
                            stop=(tg == Q - 1 and a == 1),
                        )

            for q in range(QC0):
                mms(q, lambda a, j, q=q: h0_view[:, q, a, j * NBANK : (j + 1) * NBANK])

            tg0 = QC0
            for qc in CHUNKS:
                pool = hpool if qc >= 4 else hspool
                h_t = pool.tile([P, qc, 2, D], mybir.dt.bfloat16)
                src = data_in[:, WCOLS + tg0 * TOK : WCOLS + (tg0 + qc) * TOK]
                nc.sync.dma_start(h_t[:], src.rearrange("p (q a d) -> p q a d", q=qc, a=2))
                for q in range(qc):
                    mms(tg0 + q, lambda a, j, q=q: h_t[:, q, a, j * NBANK : (j + 1) * NBANK])
                tg0 += qc

            # Per-bank epilogue: each bank's scale+store chases its own
            # accumulation group instead of waiting for all three.
            out_t = opool.tile([MM, D], mybir.dt.float32)
            for j in range(NJ):
                sl = slice(j * NBANK, (j + 1) * NBANK)
                nc.vector.tensor_scalar_mul(out_t[:, sl], psum_t[:, sl], inv_tile[:, 0:1])
                nc.sync.dma_start(out_ext[:, sl], out_t[:, sl])

    nc.compile()
    return nc


def _get_program():
    global _PROGRAM
    if _PROGRAM is None:
        _PROGRAM = _build_program()
    return _PROGRAM


def _prepare_inputs(hidden, attn_mask, sent_ids):
    """Host-side lossless recode: bf16 hi/lo split + layout permute + W."""
    import ml_dtypes

    bf16 = ml_dtypes.bfloat16

    # Last-token index per example (same semantics as the reference).
    left_padding = int(attn_mask[:, -1].sum()) == B
    seq_lengths = attn_mask.sum(axis=1) - 1  # [B]
    if left_padding:
        idx = np.full(B, S - 1, dtype=np.int64)
    else:
        idx = seq_lengths.astype(np.int64)

    hi = hidden.astype(bf16)
    lo = (hidden - hi.astype(np.float32)).astype(bf16)
    pair = np.stack([hi, lo], axis=2)  # [B, S, 2, D]
    htok = pair.reshape(B, P, Q * TOK)  # token t = p*Q + q

    w = np.zeros((B, S, MM), dtype=bf16)
    tok = np.arange(S)
    inv = np.zeros((B, MM, 1), dtype=np.float32)
    for b in range(B):
        w[b, tok, sent_ids[b]] = 1
        w[b, idx[b], M] = 1
        counts = np.bincount(sent_ids[b], minlength=M)
        inv[b, :M, 0] = (1.0 / np.maximum(counts, 1)).astype(np.float32)
        inv[b, M, 0] = 1.0
    w_part = w.reshape(B, P, WCOLS)

    data = np.concatenate([w_part, htok], axis=2)  # [B, P, WCOLS + Q*TOK]
    return np.ascontiguousarray(data), inv


def kernel(hidden, attn_mask, sent_ids, max_sents):
    global _LAST_RESULTS
    from concourse.bass_utils import run_bass_kernel_spmd

    hidden = np.ascontiguousarray(np.asarray(hidden, dtype=np.float32))
    attn_mask = np.asarray(attn_mask).astype(np.int32)
    sent_ids = np.asarray(sent_ids).astype(np.int32)
    m = int(max_sents)
    assert hidden.shape == (B, S, D) and m == M

    data, inv = _prepare_inputs(hidden, attn_mask, sent_ids)

    nc = _get_program()
    in_maps = [{"data_b": data[b], "inv_b": inv[b]} for b in range(B)]
    trace = bool(os.environ.get("KERNEL_TRACE"))
    kwargs = {}
    if trace:
        base = os.environ.get("KERNEL_TRACE_DIR")
        if base:
            import tempfile

            os.makedirs(base, exist_ok=True)
            kwargs["tmpdir"] = tempfile.mkdtemp(dir=base)
        if os.environ.get("KERNEL_TRACE_CORES"):
            kwargs["trace_cores"] = [
                int(c) for c in os.environ["KERNEL_TRACE_CORES"].split(",")
            ]
    res = run_bass_kernel_spmd(nc, in_maps, list(range(B)), trace=trace, **kwargs)
    _LAST_RESULTS = res
    out = np.stack([res.results[b]["out_b"] for b in range(B)])  # [B, MM, D]
    doc_reps = out[:, M, :].copy()
    sent_reps = out[:, :M, :].copy()
    return doc_reps, sent_reps


# revision 10
# speedup vs baseline: 1.1985x; 1.0767x over previous
"""Trainium2 Bass kernel for DecoderOnlyAspire segment-reduce problem.

Computes, for hidden [B=8, S=4096, D=1536] f32:
  - doc_reps  [B, D]    : last-token pooling (per reference semantics)
  - sent_reps [B, M, D] : per-sentence segment means (M = 24)

Strategy: data-parallel over batch across 8 NeuronCores (one example per
core; no cross-core communication).  On each core both outputs come from
PE matmuls: out[M+1, D] = W.T @ hidden_b where W [S, M+1] is the exact
{0,1} one-hot of sent_ids (col M = last-token indicator); the division
by segment count is folded into a per-partition scalar multiply on the
way out of PSUM.

Memory-roofline design: the host losslessly recodes hidden into a bf16
hi/lo pair (hi = bf16(h), lo = bf16(h - hi); |h - hi - lo| <= ~2^-18
|h|, same total bytes as fp32) packed so every DMA reads large
contiguous per-partition chunks at near line rate, and the PE runs bf16
matmuls (1 cycle/row instead of fp32's 4).  Token t lives at partition
p = t // 32, slot q = t % 32; W rows are permuted identically.  W is
inlined at the head of chunk 0 of the same stream so no small-packet
DMA ever stalls the pipeline; chunk sizes taper at the end so the PE
trail after the last DMA is short.
"""

import os

import numpy as np

B, S, D, M = 8, 4096, 1536, 24
P = 128              # SBUF partitions
Q = S // P           # 32 token slots per partition
MM = M + 1           # segment columns + doc (last-token) column
NBANK = 512          # fp32 elements per PSUM bank
NJ = D // NBANK      # 3 bank-column chunks
WCOLS = Q * MM       # 800 bf16 of inlined W per partition
TOK = 2 * D          # bf16 elements per token slot (hi+lo)

QC0 = 2                         # token slots sharing chunk 0 with W
CHUNKS = [2] * 15               # remaining token-slot chunks (fine-grained so
assert QC0 + sum(CHUNKS) == Q   # the PE tracks the DMA stream closely)

_PROGRAM = None
_LAST_RESULTS = None  # BassKernelResults of the most recent run (for test harness)


def _build_program():
    import concourse.bacc as bacc
    import concourse.tile as tile
    from concourse import mybir

    nc = bacc.Bacc("TRN2", target_bir_lowering=False, debug=False)

    # Per partition p: [ W_p (800 bf16) | token slots q=0..31, each hi|lo (2*1536) ]
    data_in = nc.declare_dram_parameter(
        "data_b", [P, WCOLS + Q * TOK], mybir.dt.bfloat16, isOutput=False
    )
    inv_in = nc.declare_dram_parameter("inv_b", [MM, 1], mybir.dt.float32, isOutput=False)
    out_ext = nc.declare_dram_parameter("out_b", [MM, D], mybir.dt.float32, isOutput=True)

    with tile.TileContext(nc) as tc:
        with (
            tc.tile_pool(name="w", bufs=1) as wpool,
            tc.tile_pool(name="h", bufs=6) as hpool,
            tc.tile_pool(name="ps", bufs=1, space="PSUM") as pspool,
            tc.tile_pool(name="o", bufs=1) as opool,
        ):
            # inv rides the ACT HWDGE ring; it is only needed at the end.
            inv_tile = wpool.tile([MM, 1], mybir.dt.float32, tag="inv")
            nc.scalar.dma_start(inv_tile[:], inv_in[:])

            # Chunk 0: W + first QC0 token slots, resident for the whole kernel.
            c0 = wpool.tile([P, WCOLS + QC0 * TOK], mybir.dt.bfloat16, tag="c0")
            nc.sync.dma_start(c0[:], data_in[:, : WCOLS + QC0 * TOK])
            w_view = c0[:, :WCOLS].rearrange("p (q m) -> p q m", q=Q)
            h0_view = c0[:, WCOLS:].rearrange("p (q a d) -> p q a d", q=QC0, a=2)

            psum_t = pspool.tile([MM, D], mybir.dt.float32)

            def mms(tg, h_slice_of):
                for a in range(2):
                    for j in range(NJ):
                        nc.tensor.matmul(
                            psum_t[:, j * NBANK : (j + 1) * NBANK],
                            w_view[:, tg, :],
                            h_slice_of(a, j),
                            start=(tg == 0 and a == 0),
                            stop=(tg == Q - 1 and a == 1),
                        )

            for q in range(QC0):
                mms(q, lambda a, j, q=q: h0_view[:, q, a, j * NBANK : (j + 1) * NBANK])

            tg0 = QC0
            for qc in CHUNKS:
                h_t = hpool.tile([P, qc, 2, D], mybir.dt.bfloat16, tag="h")
                src = data_in[:, WCOLS + tg0 * TOK : WCOLS + (tg0 + qc) * TOK]
                nc.sync.dma_start(h_t[:], src.rearrange("p (q a d) -> p q a d", q=qc, a=2))
                for q in range(qc):
                    mms(tg0 + q, lambda a, j, q=q: h_t[:, q, a, j * NBANK : (j + 1) * NBANK])
                tg0 += qc

            # Per-bank epilogue, spread across DVE and ACT so the three
            # scale+store chains run concurrently, each chasing its own
            # accumulation group.
            out_t = opool.tile([MM, D], mybir.dt.float32)
            for j in range(NJ):
                sl = slice(j * NBANK, (j + 1) * NBANK)
                if j == 1:
                    nc.scalar.activation(
                        out_t[:, sl],
                        psum_t[:, sl],
                        mybir.ActivationFunctionType.Identity,
                        scale=inv_tile[:, 0:1],
                    )
                else:
                    nc.vector.tensor_scalar_mul(
                        out_t[:, sl], psum_t[:, sl], inv_tile[:, 0:1]
                    )
                nc.sync.dma_start(out_ext[:, sl], out_t[:, sl])

    nc.compile()
    return nc


def _get_program():
    global _PROGRAM
    if _PROGRAM is None:
        _PROGRAM = _build_program()
    return _PROGRAM


def _prepare_inputs(hidden, attn_mask, sent_ids):
    """Host-side lossless recode: bf16 hi/lo split + layout permute + W."""
    import ml_dtypes

    bf16 = ml_dtypes.bfloat16

    # Last-token index per example (same semantics as the reference).
    left_padding = int(attn_mask[:, -1].sum()) == B
    seq_lengths = attn_mask.sum(axis=1) - 1  # [B]
    if left_padding:
        idx = np.full(B, S - 1, dtype=np.int64)
    else:
        idx = seq_lengths.astype(np.int64)

    hi = hidden.astype(bf16)
    lo = (hidden - hi.astype(np.float32)).astype(bf16)
    pair = np.stack([hi, lo], axis=2)  # [B, S, 2, D]
    htok = pair.reshape(B, P, Q * TOK)  # token t = p*Q + q

    w = np.zeros((B, S, MM), dtype=bf16)
    tok = np.arange(S)
    inv = np.zeros((B, MM, 1), dtype=np.float32)
    for b in range(B):
        w[b, tok, sent_ids[b]] = 1
        w[b, idx[b], M] = 1
        counts = np.bincount(sent_ids[b], minlength=M)
        inv[b, :M, 0] = (1.0 / np.maximum(counts, 1)).astype(np.float32)
        inv[b, M, 0] = 1.0
    w_part = w.reshape(B, P, WCOLS)

    data = np.concatenate([w_part, htok], axis=2)  # [B, P, WCOLS + Q*TOK]
    return np.ascontiguousarray(data), inv


def kernel(hidden, attn_mask, sent_ids, max_sents):
    global _LAST_RESULTS
    from concourse.bass_utils import run_bass_kernel_spmd

    hidden = np.ascontiguousarray(np.asarray(hidden, dtype=np.float32))
    attn_mask = np.asarray(attn_mask).astype(np.int32)
    sent_ids = np.asarray(sent_ids).astype(np.int32)
    m = int(max_sents)
    assert hidden.shape == (B, S, D) and m == M

    data, inv = _prepare_inputs(hidden, attn_mask, sent_ids)

    nc = _get_program()
    in_maps = [{"data_b": data[b], "inv_b": inv[b]} for b in range(B)]
    trace = bool(os.environ.get("KERNEL_TRACE"))
    kwargs = {}
    if trace:
        base = os.environ.get("KERNEL_TRACE_DIR")
        if base:
            import tempfile

            os.makedirs(base, exist_ok=True)
            kwargs["tmpdir"] = tempfile.mkdtemp(dir=base)
        if os.environ.get("KERNEL_TRACE_CORES"):
            kwargs["trace_cores"] = [
                int(c) for c in os.environ["KERNEL_TRACE_CORES"].split(",")
            ]
    res = run_bass_kernel_spmd(nc, in_maps, list(range(B)), trace=trace, **kwargs)
    _LAST_RESULTS = res
    out = np.stack([res.results[b]["out_b"] for b in range(B)])  # [B, MM, D]
    doc_reps = out[:, M, :].copy()
    sent_reps = out[:, :M, :].copy()
    return doc_reps, sent_reps
